# revision 23
# baseline (speedup 1.0000x reference)
"""A3TGCN (cat-1) Trainium2 kernel, data-parallel over batch on 8 NeuronCores.

Math restructuring (exact, no approximation):
  - A3TGCN2 passes H=None every period, so per-period hidden state is
    H_t = (1 - Z_t) * tanh_t with Z_t = sigmoid(lin_z(gcn_z(x_t))),
    i.e. H_t depends only on x_t.  x_t takes just 3 values over t:
    ad (t < los-1), dis (t == los-1), 0 (t > los-1).  The attention
    einsum over t therefore collapses to
        after_gnn = c_ad*H(ad) + c_dis*H(dis) + c_zero*H(0)
    with per-batch scalars c_* = sums of softmax(attention) segments.
  - GCNConv + Linear fold into one [128,128] weight:
        A_pre = S @ x @ (conv_w @ lin_w[:64]) + (conv_b @ lin_w[:64] + lin_b)
    where S = D^-1/2 (A + I) D^-1/2 (dense 512x512, shared by all graphs).
  - tanh(v) = 2*sigmoid(2v) - 1 lets one 128-partition sigmoid handle both
    gates (z rows get scale 1, h rows scale 2, biases pre-scaled).
  - per graph, T = 2*Qh-1 and sum_n H = sum T - sum Z*T come from two DVE
    ops with free accumulators; the H(0) branch folds into a host constant.

Device-side layout choices (from neuron-profile traces):
  - DMA issue costs ~700ns each on the issuing engine, so inputs are
    host-permuted into partition-major blocks and shipped as 4 large 2D
    DMAs (S^T, x in two halves, one packed const tile) instead of 23.
  - f32 matmuls lower to two hardware passes; all matmul operands are
    bf16 (PSUM accumulation stays f32).
  - q stays f32: tanh comes from 2*sigmoid-1, so q ~= 0.5 + tiny signal
    and low-precision q would wipe the signal out.

Per core: 4 batches x {ad, dis} = 8 graphs of 512 nodes.  No collectives.
"""

import numpy as np

B = 32
R = 1024
C = 8
D = 16
N = 512
T = 37
HID = 64
F = C * D  # 128
NCORES = 8
BPC = B // NCORES  # 4 batches per core
G = 2 * BPC        # 8 graphs per core

# packed const tile columns (f32): biasp | scalep | cb1 | cw1 | ctile | pz | cw2 | cb2
_C_BIAS = 0
_C_SCALE = 1
_C_CB1 = 2
_C_CW1 = 3
_C_CTILE = _C_CW1 + 2 * HID          # 131
_C_PZ = _C_CTILE + G                 # 139
_C_CW2 = _C_PZ + BPC                 # 143
_C_CB2 = _C_CW2 + 2                  # 145
_C_TOT = _C_CB2 + 1                  # 146

_CACHE = {}


def _get_nc(dt_name):
    key = ("nc", dt_name)
    if key in _CACHE:
        return _CACHE[key]

    import concourse.mybir as mybir
    import concourse.tile as tile
    from concourse import bacc

    f32 = mybir.dt.float32
    f16 = mybir.dt.float16
    dtx = getattr(mybir.dt, dt_name)

    nc = bacc.Bacc()
    x_e = nc.declare_dram_parameter("x", [128, G * 4 * F], dtx, isOutput=False)
    st_e = nc.declare_dram_parameter("st", [128, 4 * N], dtx, isOutput=False)
    w_e = nc.declare_dram_parameter("w", [F, 2 * HID], dtx, isOutput=False)
    cst_e = nc.declare_dram_parameter("cst", [128, _C_TOT], f32, isOutput=False)
    out_e = nc.declare_dram_parameter("out", [2, BPC], f32, isOutput=True)

    AF = mybir.ActivationFunctionType
    ALU = mybir.AluOpType

    with tile.TileContext(nc) as tc:
        with (
            tc.tile_pool(name="const", bufs=1) as cpool,
            tc.tile_pool(name="work", bufs=3) as wpool,
            tc.tile_pool(name="psumy", bufs=3, space="PSUM") as ppooly,
            tc.tile_pool(name="psum", bufs=2, space="PSUM") as ppool,
            tc.tile_pool(name="psum1", bufs=1, space="PSUM") as ppool1,
        ):
            # Few large DMAs (issue cost ~700ns each), spread across idle
            # engine queues so issue happens in parallel.  gpsimd is avoided:
            # its SWDGE path inserts a ~7us drain that gates the whole head.
            # Split transfers per consumer (one DMA queue moves only
            # ~70GB/s, a 512KB block takes ~7us) and interleave issue on the
            # two HWDGE issuers in first-needed order.
            st = [cpool.tile([128, N], dtx, tag=f"st{k}", name=f"st{k}") for k in range(4)]
            xt = [cpool.tile([128, 4 * F], dtx, tag=f"x{g}", name=f"x{g}") for g in range(G)]
            w = cpool.tile([F, 2 * HID], dtx)
            cst = cpool.tile([128, _C_TOT], f32)

            nc.sync.dma_start(out=st[0], in_=st_e[:, 0:N])
            nc.scalar.dma_start(out=st[1], in_=st_e[:, N:2 * N])
            nc.sync.dma_start(out=xt[0], in_=x_e[:, 0:4 * F])
            nc.scalar.dma_start(out=st[2], in_=st_e[:, 2 * N:3 * N])
            nc.sync.dma_start(out=st[3], in_=st_e[:, 3 * N:4 * N])
            nc.scalar.dma_start(out=xt[1], in_=x_e[:, 4 * F:8 * F])
            nc.sync.dma_start(out=w, in_=w_e[:])
            nc.scalar.dma_start(out=cst, in_=cst_e[:])
            for g in range(2, G):
                eng = nc.sync if g % 2 == 0 else nc.scalar
                eng.dma_start(out=xt[g], in_=x_e[:, g * 4 * F:(g + 1) * 4 * F])

            biasp = cst[:, _C_BIAS:_C_BIAS + 1]
            scalep = cst[:, _C_SCALE:_C_SCALE + 1]
            cb1 = cst[:, _C_CB1:_C_CB1 + 1]
            cw1 = cst[0:HID, _C_CW1:_C_CW1 + 2 * HID]
            ctile = cst[0:HID, _C_CTILE:_C_CTILE + G]
            pz = cst[0:HID, _C_PZ:_C_PZ + BPC]
            cw2 = cst[:, _C_CW2:_C_CW2 + 2]
            cb2 = cst[0:2, _C_CB2:_C_CB2 + 1]

            accT = cpool.tile([HID, G], f32)
            accZ = cpool.tile([HID, G], f32)

            # Warm the PE HAM state during the input-DMA window with matmuls
            # on a zeroed scratch tile (results never read).
            wsc_in = cpool.tile([128, N], dtx)
            nc.vector.memset(wsc_in, 0.0)
            pwu = ppool1.tile([128, N], f32, tag="pwu")
            for _ in range(8):
                nc.tensor.matmul(pwu, wsc_in[:, 0:128], wsc_in,
                                 start=True, stop=True)

            for g in range(G):
                # y^T = (S @ x_g)^T : accumulate 4 node-chunks
                py = ppooly.tile([128, N], f32, tag="py")
                for k in range(4):
                    nc.tensor.matmul(py, xt[g][:, k * F:(k + 1) * F], st[k],
                                     start=(k == 0), stop=(k == 3))
                ysb = wpool.tile([128, N], dtx, tag="ysb")
                # alternate the PSUM->SBUF cast between ACT and DVE to balance
                if g % 2 == 0:
                    nc.vector.tensor_copy(ysb, py)
                else:
                    nc.scalar.copy(ysb, py)
                # A^T = W_all^T @ y^T  (rows 0:64 = z gate, 64:128 = h gate)
                pa = ppool.tile([128, N], f32, tag="pa")
                nc.tensor.matmul(pa, w, ysb, start=True, stop=True)
                # Q = sigmoid(scale*A + bias)
                q = wpool.tile([128, N], f32, tag="q")
                nc.scalar.activation(q, pa, AF.Sigmoid, bias=biasp, scale=scalep)
                # DVE two-SBUF-input ops need equal base partitions, so shift
                # the h half down with a SBUF->SBUF DMA, then accumulate
                # sum_n T and sum_n (-Z*T) via the ops' accumulators.
                qh = wpool.tile([HID, N], f32, tag="qh")
                nc.sync.dma_start(out=qh, in_=q[HID:128, :])
                # tT = qh - 0.5 = T/2; accT = reduce_add(tT, init=0) = sum T/2
                tT = wpool.tile([HID, N], f32, tag="tT")
                nc.vector.tensor_scalar(
                    out=tT, in0=qh, scalar1=0.5, scalar2=0.0,
                    op0=ALU.subtract, op1=ALU.add, accum_out=accT[:, g:g + 1])
                # sc = (-2*Z)*tT = -Z*T; accZ = sum_n -Z*T
                sc = wpool.tile([HID, N], f32, tag="sc")
                nc.vector.scalar_tensor_tensor(
                    out=sc, in0=q[0:HID, :], scalar=-2.0, in1=tT,
                    op0=ALU.mult, op1=ALU.mult, accum_out=accZ[:, g:g + 1])

            # sum_n H = 2*(sum T/2) + sum(-Z*T)  (no large-term cancellation)
            v = cpool.tile([HID, G], f32)
            nc.vector.scalar_tensor_tensor(out=v, in0=accT, scalar=2.0, in1=accZ,
                                           op0=ALU.mult, op1=ALU.add)
            wsc = cpool.tile([HID, G], f32)
            nc.vector.tensor_mul(wsc, v, ctile)
            pooled = cpool.tile([HID, BPC], f32)
            nc.vector.tensor_add(pooled, wsc[:, 0:BPC], wsc[:, BPC:G])
            nc.vector.tensor_add(pooled, pooled, pz)

            # classifier: h1 = relu(cls_w1^T pooled + b1); out = cls_w2^T h1 + b2
            ph1 = ppool1.tile([2 * HID, BPC], f32, tag="ph1")
            nc.tensor.matmul(ph1, cw1, pooled, start=True, stop=True)
            h1 = cpool.tile([2 * HID, BPC], f32)
            nc.scalar.activation(h1, ph1, AF.Relu, bias=cb1)
            po = ppool1.tile([2, BPC], f32, tag="po")
            nc.tensor.matmul(po, cw2, h1, start=True, stop=True)
            osb = cpool.tile([2, BPC], f32)
            nc.vector.tensor_scalar_add(osb, po, cb2)
            nc.sync.dma_start(out=out_e[:], in_=osb)

    nc.compile()
    _CACHE[key] = nc
    return nc


def _host_prep(inputs, np_dtype):
    x_batch = np.asarray(inputs["x_batch"])
    LOS = np.asarray(inputs["LOS_batch"])
    ad_idx = np.asarray(inputs["ad_col_index"])
    dis_idx = np.asarray(inputs["dis_col_index"])
    edges = np.asarray(inputs["template_edge_index"])
    emb = np.asarray(inputs["emb_tables"], np.float32)

    # entity embedding + row select (index-select preprocessing)
    xe = emb[np.arange(C)[None, None, :], x_batch].reshape(B, R, F)
    ad = xe[:, ad_idx]
    dis = xe[:, dis_idx]

    # dense S^T with self loops + symmetric norm (multi-edges accumulate)
    src, dst = edges[0], edges[1]
    deg = np.zeros(N, np.float64)
    np.add.at(deg, dst, 1.0)
    deg += 1.0
    dinv = deg ** -0.5
    S = np.zeros((N, N), np.float64)
    np.add.at(S, (dst, src), dinv[dst] * dinv[src])
    S[np.arange(N), np.arange(N)] += dinv * dinv
    ST = S.T.astype(np.float32)
    # partition-major: stp[p, k*N:(k+1)*N] = S^T[k*128+p, :]
    stp = np.ascontiguousarray(
        ST.reshape(4, 128, N).transpose(1, 0, 2).reshape(128, 4 * N)).astype(np_dtype)

    # fold conv+lin weights/biases per gate (r gate is dead: H_prev = 0)
    lz = np.asarray(inputs["lin_w_z"], np.float32)[:HID]
    lh = np.asarray(inputs["lin_w_h"], np.float32)[:HID]
    Wz = np.asarray(inputs["conv_w_z"], np.float32) @ lz
    Wh = np.asarray(inputs["conv_w_h"], np.float32) @ lh
    W_all = np.concatenate([Wz, Wh], axis=1).astype(np_dtype)
    bz = np.asarray(inputs["conv_b_z"], np.float32) @ lz + np.asarray(inputs["lin_b_z"], np.float32)
    bh = np.asarray(inputs["conv_b_h"], np.float32) @ lh + np.asarray(inputs["lin_b_h"], np.float32)

    # temporal-collapse coefficients
    att = np.asarray(inputs["attention"], np.float64)
    p = np.exp(att - att.max())
    p /= p.sum()
    c_ad = np.array([p[: l - 1].sum() for l in LOS])
    c_dis = p[LOS - 1]
    c_zero = np.array([p[l:].sum() for l in LOS])

    # H(0) branch: gcn(0) = conv_b, so pre-act = bz / bh exactly
    z0 = 1.0 / (1.0 + np.exp(-bz.astype(np.float64)))
    Hz0 = (1.0 - z0) * np.tanh(bh.astype(np.float64))

    in_maps = []
    for c in range(NCORES):
        bs = range(c * BPC, (c + 1) * BPC)
        xg = np.concatenate([ad[c * BPC:(c + 1) * BPC], dis[c * BPC:(c + 1) * BPC]],
                            axis=0)  # [G, 512, 128]
        # partition-major blocks: xp[p, (g*4+k)*F:(...)+F] = x_g[k*128+p, :]
        xp = np.ascontiguousarray(
            xg.reshape(G, 4, 128, F).transpose(2, 0, 1, 3).reshape(128, G * 4 * F)
        ).astype(np_dtype)

        cstt = np.zeros((128, _C_TOT), np.float32)
        cstt[:, _C_BIAS] = np.concatenate([bz, 2.0 * bh])
        cstt[:, _C_SCALE] = np.concatenate([np.ones(HID), 2.0 * np.ones(HID)])
        cstt[:, _C_CB1] = np.asarray(inputs["cls_b1"], np.float32)
        cstt[0:HID, _C_CW1:_C_CW1 + 2 * HID] = np.asarray(inputs["cls_w1"], np.float32)
        for j, b in enumerate(bs):
            cstt[0:HID, _C_CTILE + j] = c_ad[b] / N
            cstt[0:HID, _C_CTILE + BPC + j] = c_dis[b] / N
            cstt[0:HID, _C_PZ + j] = c_zero[b] * Hz0
        cstt[:, _C_CW2:_C_CW2 + 2] = np.asarray(inputs["cls_w2"], np.float32)
        cstt[0:2, _C_CB2] = np.asarray(inputs["cls_b2"], np.float32)

        in_maps.append({"x": xp, "st": stp, "w": W_all, "cst": cstt})
    return in_maps


DT_NAME = "bfloat16"


def _np_dt(name):
    if name == "bfloat16":
        import ml_dtypes
        return ml_dtypes.bfloat16
    return np.float32


def kernel(**inputs):
    from concourse.bass_utils import run_bass_kernel_spmd

    nc = _get_nc(DT_NAME)
    in_maps = _host_prep(inputs, _np_dt(DT_NAME))
    res = run_bass_kernel_spmd(nc, in_maps, core_ids=list(range(NCORES)))
    out = np.empty((B, 2), np.float32)
    for c in range(NCORES):
        out[c * BPC:(c + 1) * BPC, :] = res.results[c]["out"].T
    return out


# revision 24
# speedup vs baseline: 1.0730x; 1.0730x over previous
"""A3TGCN (cat-1) Trainium2 kernel, data-parallel over batch on 8 NeuronCores.

Math restructuring (exact, no approximation):
  - A3TGCN2 passes H=None every period, so per-period hidden state is
    H_t = (1 - Z_t) * tanh_t with Z_t = sigmoid(lin_z(gcn_z(x_t))),
    i.e. H_t depends only on x_t.  x_t takes just 3 values over t:
    ad (t < los-1), dis (t == los-1), 0 (t > los-1).  The attention
    einsum over t therefore collapses to
        after_gnn = c_ad*H(ad) + c_dis*H(dis) + c_zero*H(0)
    with per-batch scalars c_* = sums of softmax(attention) segments.
  - GCNConv + Linear fold into one [128,128] weight:
        A_pre = S @ x @ (conv_w @ lin_w[:64]) + (conv_b @ lin_w[:64] + lin_b)
    where S = D^-1/2 (A + I) D^-1/2 (dense 512x512, shared by all graphs).
  - tanh(v) = 2*sigmoid(2v) - 1 lets one 128-partition sigmoid handle both
    gates (z rows get scale 1, h rows scale 2, biases pre-scaled).
  - per graph, T = 2*Qh-1 and sum_n H = sum T - sum Z*T come from two DVE
    ops with free accumulators; the H(0) branch folds into a host constant.

Device-side layout choices (from neuron-profile traces):
  - DMA issue costs ~700ns each on the issuing engine, so inputs are
    host-permuted into partition-major blocks and shipped as 4 large 2D
    DMAs (S^T, x in two halves, one packed const tile) instead of 23.
  - f32 matmuls lower to two hardware passes; all matmul operands are
    bf16 (PSUM accumulation stays f32).
  - q stays f32: tanh comes from 2*sigmoid-1, so q ~= 0.5 + tiny signal
    and low-precision q would wipe the signal out.

Per core: 4 batches x {ad, dis} = 8 graphs of 512 nodes.  No collectives.
"""

import numpy as np

B = 32
R = 1024
C = 8
D = 16
N = 512
T = 37
HID = 64
F = C * D  # 128
NCORES = 8
BPC = B // NCORES  # 4 batches per core
G = 2 * BPC        # 8 graphs per core

# packed const tile columns (f32): biasp | scalep | cb1 | cw1 | ctile | pz | cw2 | cb2
_C_BIAS = 0
_C_SCALE = 1
_C_CB1 = 2
_C_CW1 = 3
_C_CTILE = _C_CW1 + 2 * HID          # 131
_C_PZ = _C_CTILE + G                 # 139
_C_CW2 = _C_PZ + BPC                 # 143
_C_CB2 = _C_CW2 + 2                  # 145
_C_TOT = _C_CB2 + 1                  # 146

_CACHE = {}


def _get_nc(dt_name):
    key = ("nc", dt_name)
    if key in _CACHE:
        return _CACHE[key]

    import concourse.mybir as mybir
    import concourse.tile as tile
    from concourse import bacc

    f32 = mybir.dt.float32
    f16 = mybir.dt.float16
    dtx = getattr(mybir.dt, dt_name)

    nc = bacc.Bacc()
    x_e = nc.declare_dram_parameter("x", [128, G * 4 * F], dtx, isOutput=False)
    st_e = nc.declare_dram_parameter("st", [128, 4 * N], dtx, isOutput=False)
    w_e = nc.declare_dram_parameter("w", [F, 2 * HID], dtx, isOutput=False)
    cst_e = nc.declare_dram_parameter("cst", [128, _C_TOT], f32, isOutput=False)
    out_e = nc.declare_dram_parameter("out", [2, BPC], f32, isOutput=True)

    AF = mybir.ActivationFunctionType
    ALU = mybir.AluOpType

    with tile.TileContext(nc) as tc:
        with (
            tc.tile_pool(name="const", bufs=1) as cpool,
            tc.tile_pool(name="work", bufs=4) as wpool,
            tc.tile_pool(name="psumy", bufs=3, space="PSUM") as ppooly,
            tc.tile_pool(name="psum", bufs=2, space="PSUM") as ppool,
            tc.tile_pool(name="psum1", bufs=1, space="PSUM") as ppool1,
        ):
            # Few large DMAs (issue cost ~700ns each), spread across idle
            # engine queues so issue happens in parallel.  gpsimd is avoided:
            # its SWDGE path inserts a ~7us drain that gates the whole head.
            # Split transfers per consumer (one DMA queue moves only
            # ~70GB/s, a 512KB block takes ~7us) and interleave issue on the
            # two HWDGE issuers in first-needed order.
            st = [cpool.tile([128, N], dtx, tag=f"st{k}", name=f"st{k}") for k in range(4)]
            xt = [cpool.tile([128, 4 * F], dtx, tag=f"x{g}", name=f"x{g}") for g in range(G)]
            w = cpool.tile([F, 2 * HID], dtx)
            cst = cpool.tile([128, _C_TOT], f32)

            nc.sync.dma_start(out=st[0][0:64, :], in_=st_e[0:64, 0:N])
            nc.scalar.dma_start(out=st[0][64:128, :], in_=st_e[64:128, 0:N])
            nc.sync.dma_start(out=xt[0][0:64, :], in_=x_e[0:64, 0:4 * F])
            nc.scalar.dma_start(out=xt[0][64:128, :], in_=x_e[64:128, 0:4 * F])
            nc.sync.dma_start(out=st[1], in_=st_e[:, N:2 * N])
            nc.scalar.dma_start(out=st[2], in_=st_e[:, 2 * N:3 * N])
            nc.sync.dma_start(out=st[3], in_=st_e[:, 3 * N:4 * N])
            nc.scalar.dma_start(out=cst, in_=cst_e[:])
            nc.sync.dma_start(out=w, in_=w_e[:])
            for g in range(1, G):
                eng = nc.scalar if g % 2 == 0 else nc.sync
                eng.dma_start(out=xt[g], in_=x_e[:, g * 4 * F:(g + 1) * 4 * F])

            biasp = cst[:, _C_BIAS:_C_BIAS + 1]
            scalep = cst[:, _C_SCALE:_C_SCALE + 1]
            cb1 = cst[:, _C_CB1:_C_CB1 + 1]
            cw1 = cst[0:HID, _C_CW1:_C_CW1 + 2 * HID]
            ctile = cst[0:HID, _C_CTILE:_C_CTILE + G]
            pz = cst[0:HID, _C_PZ:_C_PZ + BPC]
            cw2 = cst[:, _C_CW2:_C_CW2 + 2]
            cb2 = cst[0:2, _C_CB2:_C_CB2 + 1]

            accq = cpool.tile([128, G], f32)
            szh = cpool.tile([HID, G], f32)

            # Warm the PE HAM state during the input-DMA window with matmuls
            # on a zeroed scratch tile (results never read).
            wsc_in = cpool.tile([128, N], dtx)
            nc.vector.memset(wsc_in, 0.0)
            pwu = ppool1.tile([128, N], f32, tag="pwu")
            for _ in range(6):
                nc.tensor.matmul(pwu, wsc_in[:, 0:128], wsc_in,
                                 start=True, stop=True)

            for g in range(G):
                # y^T = (S @ x_g)^T : accumulate 4 node-chunks
                py = ppooly.tile([128, N], f32, tag="py")
                for k in range(4):
                    nc.tensor.matmul(py, xt[g][:, k * F:(k + 1) * F], st[k],
                                     start=(k == 0), stop=(k == 3))
                ysb = wpool.tile([128, N], dtx, tag="ysb")
                # split the PSUM->SBUF cast between DVE and ACT
                nc.vector.tensor_copy(ysb[0:64, :], py[0:64, :])
                nc.scalar.copy(ysb[64:128, :], py[64:128, :])
                # A^T = W_all^T @ y^T  (rows 0:64 = z gate, 64:128 = h gate)
                pa = ppool.tile([128, N], f32, tag="pa")
                nc.tensor.matmul(pa, w, ysb, start=True, stop=True)
                # Q = sigmoid(scale*A + bias); accum_out -> column of per-hid
                # sums (rows 0:64 = sum Z, rows 64:128 = sum Qh)
                q = wpool.tile([128, N], f32, tag="q")
                nc.scalar.activation(q, pa, AF.Sigmoid, bias=biasp, scale=scalep,
                                     accum_out=accq[:, g:g + 1])
                # DVE two-SBUF-input ops need equal base partitions, so shift
                # the h half down with a SBUF->SBUF DMA, then one stt gives
                # sum_n Z*Qh.
                qh = wpool.tile([HID, N], f32, tag="qh")
                nc.sync.dma_start(out=qh, in_=q[HID:128, :])
                sc = wpool.tile([HID, N], f32, tag="sc")
                nc.vector.scalar_tensor_tensor(
                    out=sc, in0=q[0:HID, :], scalar=1.0, in1=qh,
                    op0=ALU.mult, op1=ALU.mult, accum_out=szh[:, g:g + 1])

            # sum_n H = accq_z + 2*(accq_h - szh) - N  (the -N folds into pz)
            accqh = cpool.tile([HID, G], f32)
            nc.sync.dma_start(out=accqh, in_=accq[HID:128, :])
            d = cpool.tile([HID, G], f32)
            nc.vector.tensor_sub(d, accqh, szh)
            v = cpool.tile([HID, G], f32)
            nc.vector.scalar_tensor_tensor(out=v, in0=d, scalar=2.0,
                                           in1=accq[0:HID, :],
                                           op0=ALU.mult, op1=ALU.add)
            wsc = cpool.tile([HID, G], f32)
            nc.vector.tensor_mul(wsc, v, ctile)
            pooled = cpool.tile([HID, BPC], f32)
            nc.vector.tensor_add(pooled, wsc[:, 0:BPC], wsc[:, BPC:G])
            nc.vector.tensor_add(pooled, pooled, pz)

            # classifier: h1 = relu(cls_w1^T pooled + b1); out = cls_w2^T h1 + b2
            ph1 = ppool1.tile([2 * HID, BPC], f32, tag="ph1")
            nc.tensor.matmul(ph1, cw1, pooled, start=True, stop=True)
            h1 = cpool.tile([2 * HID, BPC], f32)
            nc.scalar.activation(h1, ph1, AF.Relu, bias=cb1)
            po = ppool1.tile([2, BPC], f32, tag="po")
            nc.tensor.matmul(po, cw2, h1, start=True, stop=True)
            osb = cpool.tile([2, BPC], f32)
            nc.vector.tensor_scalar_add(osb, po, cb2)
            nc.sync.dma_start(out=out_e[:], in_=osb)

    nc.compile()
    _CACHE[key] = nc
    return nc


def _host_prep(inputs, np_dtype):
    x_batch = np.asarray(inputs["x_batch"])
    LOS = np.asarray(inputs["LOS_batch"])
    ad_idx = np.asarray(inputs["ad_col_index"])
    dis_idx = np.asarray(inputs["dis_col_index"])
    edges = np.asarray(inputs["template_edge_index"])
    emb = np.asarray(inputs["emb_tables"], np.float32)

    # entity embedding + row select (index-select preprocessing)
    xe = emb[np.arange(C)[None, None, :], x_batch].reshape(B, R, F)
    ad = xe[:, ad_idx]
    dis = xe[:, dis_idx]

    # dense S^T with self loops + symmetric norm (multi-edges accumulate)
    src, dst = edges[0], edges[1]
    deg = np.zeros(N, np.float64)
    np.add.at(deg, dst, 1.0)
    deg += 1.0
    dinv = deg ** -0.5
    S = np.zeros((N, N), np.float64)
    np.add.at(S, (dst, src), dinv[dst] * dinv[src])
    S[np.arange(N), np.arange(N)] += dinv * dinv
    ST = S.T.astype(np.float32)
    # partition-major: stp[p, k*N:(k+1)*N] = S^T[k*128+p, :]
    stp = np.ascontiguousarray(
        ST.reshape(4, 128, N).transpose(1, 0, 2).reshape(128, 4 * N)).astype(np_dtype)

    # fold conv+lin weights/biases per gate (r gate is dead: H_prev = 0)
    lz = np.asarray(inputs["lin_w_z"], np.float32)[:HID]
    lh = np.asarray(inputs["lin_w_h"], np.float32)[:HID]
    Wz = np.asarray(inputs["conv_w_z"], np.float32) @ lz
    Wh = np.asarray(inputs["conv_w_h"], np.float32) @ lh
    W_all = np.concatenate([Wz, Wh], axis=1).astype(np_dtype)
    bz = np.asarray(inputs["conv_b_z"], np.float32) @ lz + np.asarray(inputs["lin_b_z"], np.float32)
    bh = np.asarray(inputs["conv_b_h"], np.float32) @ lh + np.asarray(inputs["lin_b_h"], np.float32)

    # temporal-collapse coefficients
    att = np.asarray(inputs["attention"], np.float64)
    p = np.exp(att - att.max())
    p /= p.sum()
    c_ad = np.array([p[: l - 1].sum() for l in LOS])
    c_dis = p[LOS - 1]
    c_zero = np.array([p[l:].sum() for l in LOS])

    # H(0) branch: gcn(0) = conv_b, so pre-act = bz / bh exactly
    z0 = 1.0 / (1.0 + np.exp(-bz.astype(np.float64)))
    Hz0 = (1.0 - z0) * np.tanh(bh.astype(np.float64))

    in_maps = []
    for c in range(NCORES):
        bs = range(c * BPC, (c + 1) * BPC)
        xg = np.concatenate([ad[c * BPC:(c + 1) * BPC], dis[c * BPC:(c + 1) * BPC]],
                            axis=0)  # [G, 512, 128]
        # partition-major blocks: xp[p, (g*4+k)*F:(...)+F] = x_g[k*128+p, :]
        xp = np.ascontiguousarray(
            xg.reshape(G, 4, 128, F).transpose(2, 0, 1, 3).reshape(128, G * 4 * F)
        ).astype(np_dtype)

        cstt = np.zeros((128, _C_TOT), np.float32)
        cstt[:, _C_BIAS] = np.concatenate([bz, 2.0 * bh])
        cstt[:, _C_SCALE] = np.concatenate([np.ones(HID), 2.0 * np.ones(HID)])
        cstt[:, _C_CB1] = np.asarray(inputs["cls_b1"], np.float32)
        cstt[0:HID, _C_CW1:_C_CW1 + 2 * HID] = np.asarray(inputs["cls_w1"], np.float32)
        for j, b in enumerate(bs):
            cstt[0:HID, _C_CTILE + j] = c_ad[b] / N
            cstt[0:HID, _C_CTILE + BPC + j] = c_dis[b] / N
            cstt[0:HID, _C_PZ + j] = c_zero[b] * Hz0 - (c_ad[b] + c_dis[b])
        cstt[:, _C_CW2:_C_CW2 + 2] = np.asarray(inputs["cls_w2"], np.float32)
        cstt[0:2, _C_CB2] = np.asarray(inputs["cls_b2"], np.float32)

        in_maps.append({"x": xp, "st": stp, "w": W_all, "cst": cstt})
    return in_maps


DT_NAME = "bfloat16"


def _np_dt(name):
    if name == "bfloat16":
        import ml_dtypes
        return ml_dtypes.bfloat16
    return np.float32


def kernel(**inputs):
    from concourse.bass_utils import run_bass_kernel_spmd

    nc = _get_nc(DT_NAME)
    in_maps = _host_prep(inputs, _np_dt(DT_NAME))
    res = run_bass_kernel_spmd(nc, in_maps, core_ids=list(range(NCORES)))
    out = np.empty((B, 2), np.float32)
    for c in range(NCORES):
        out[c * BPC:(c + 1) * BPC, :] = res.results[c]["out"].T
    return out
